# revision 77
# baseline (speedup 1.0000x reference)
"""Trainium2 Bass kernel for nn_Adaptive_dilatedConv (dense_cnn), v2.

Reference computation (per image):
  logits = einsum('bchw,kc->bkhw', x, attn_w) + attn_b        # [B,3,H,W]
  attn   = softmax(logits, axis=1)
  convs_k = depthwise3x3(x, dw_w[k], dilation d_k) + dw_b[k]  # [B,C,H,W] x3
  fused  = sum_k convs_k * (attn_k + 1)
  out    = einsum('bchw,oc->bohw', fused, out_w) + out_b

Distribution: data parallel over batch (16 images over 8 cores).

Per-core schedule (all engines balanced, two images software-pipelined):
  - x arrives pre-padded from the host as [2, 128, 74*74] bf16 frames, plus
    fp8(e4m3) copies (and fp8 residual copies) of the same frames.
  - attention: per-hw-tile matmuls (lhsT = padded-frame interior views) give
    transposed logits [hw, (j,k)]; softmax in that layout with exp(attn_b)
    folded as host immediates; TensorE transpose; k-major DRAM bounce gives
    (1 + attn_k) as three [128, 4096] partition-broadcast bf16 tensors
    ((1+attn_1) pre-scaled by 1/64 to undo the fp8 weight scaling below).
  - branch d=2 runs ENTIRELY on the TensorEngine in fp8 DoubleRow mode:
    host precomputes g[c,t,o] = out_w[o,c] * dw_w[1][c,t] * 64, split into
    e4m3 main + e5m2 residual; per 4-row output block one PSUM chain
    accumulates 9 taps x (main, w-resid[, x-resid]) DoubleRow matmuls over
    contiguous 296-element runs of the padded frame (74-col geometry keeps
    every tap view one contiguous run; pad columns are computed and
    discarded).  This fuses conv1 AND its 1x1 out-conv at 1/4 the bf16
    matmul cost; the conv bias rides the gw combine as a per-partition
    scalar.
  - branches d=1 and d=5 are elementwise, spread over three engines:
    ScalarE activation-Copy quarter-products with DVE adds, and DVE
    half-products with gpsimd tensor_tensor adds (gpsimd lacks the fused
    scalar_tensor_tensor opcode on hardware).
  - (1 + attn_k) applied in place on DVE; out conv P_a = OW @ m_k
    accumulates both weighted branches in PSUM (4 matmuls per 256-col
    block), interleaved into the NEXT image's fp8 chain window so the
    TensorEngine never drains.
  - combines split by hardware rules (GPSIMD may not access PSUM):
    two fp8 chains share one 2-bank PSUM tile and ACT/DVE drain them in a
    single paired op gw = P_g/64 + q1, gpsimd applies (1+attn_1) in SBUF,
    DVE fuses osb = (P_a + out_b) + gw into f32 staging eighths;
    contiguous DMAs store the output.  The second image
    skips the fp8 x-residual matmuls (~1% output error, 2x under the
    tolerance) which unblocks its chain window and shortens the tail.
  - emission is explicitly interleaved (chains / taps / combines / next
    image's attention) because each engine executes its stream in order.
"""

import sys
from contextlib import ExitStack

import numpy as np

sys.path.insert(0, "/opt/trn_rl_repo")

import concourse.bass as bass  # noqa: E402
import concourse.bacc as bacc  # noqa: E402
import concourse.mybir as mybir  # noqa: E402
import concourse.tile as tile  # noqa: E402
from concourse.masks import make_identity  # noqa: E402

F32 = mybir.dt.float32
BF16 = mybir.dt.bfloat16
FP8 = mybir.dt.float8e4
FP8E5 = mybir.dt.float8e5

N_CORES = 8
B, C, H, W = 16, 256, 64, 64
PB = B // N_CORES
PAD = 5
WP = W + 2 * PAD  # 74
FW = WP * WP  # 5476
FWE = FW + 16  # frame + tail pad so d=5 corner runs stay in-bounds
HW = H * W  # 4096
GSCALE = 64.0
GINV = 1.0 / GSCALE
NBLK = 16  # 4-row blocks per ok-chunk
BR = 4  # rows per block
RUN = BR * WP  # 296-element contiguous run per DoubleRow rhs half

# taps (of branch d=2) that get the fp8 x-residual correction matmul
XCOMP_TAPS = ()
DIAG = True  # branch d=5 ck=1 on PE via diagonal-matmul chains

AluOp = mybir.AluOpType
ActFn = mybir.ActivationFunctionType
PerfMode = mybir.MatmulPerfMode

DILS = {0: 1, 1: 5}  # elementwise branch index -> dilation

# ---- tap routing (per image) ----
# acc (ki, ck) first-op engine: d = DVE tensor_scalar(+bias), a = ACT Identity
FIRST_ROUTE = {(0, 0): "d", (1, 0): "d", (0, 1): "d", (1, 1): "d"}
# (ki, ck, t) -> (product engine, add engine); adds: ck0 accs on DVE (2x tt),
# ck1 accs on Pool; products: DVE (4x ts) early taps, ACT the tail taps --
# each acc chain accumulates its DVE-sourced taps FIRST so no add ever
# waits on the slower ACT product stream at the head of the chain
TAP_ROUTE = {}
for _ki in range(2):
    for _ck in range(2):
        for _t in range(1, 9):
            # ck0 chains: DVE prods+adds t1-5, ACT prods + Pool adds t6-8;
            # ck1 chains: Pool adds throughout, ACT prods early (t1-5) so
            # Pool starts at step 0, DVE prods for the tail
            if _ck == 0:
                _add = "d" if _t <= 5 else "g"
                _prod = "d" if _t <= 5 else "a"
            else:
                _add = "g"
                _prod = "a" if _t <= 5 else "d"
            TAP_ROUTE[(_ki, _ck, _t)] = (_prod, _add)


def build_bass(u_vals, reps=1):
    nc = bacc.Bacc()

    xpe_d = nc.declare_dram_parameter("xpe", [PB, 2, 128, FWE], BF16,
                                      isOutput=False)
    x8_d = nc.declare_dram_parameter("x8", [PB, 128, 2, FW], FP8,
                                     isOutput=False)
    xr8_d = nc.declare_dram_parameter("xr8", [PB, 128, 2, FW], FP8,
                                      isOutput=False)
    g8_d = nc.declare_dram_parameter("g8", [128, 2, 9, 2, 128], FP8,
                                     isOutput=False)
    gr8_d = nc.declare_dram_parameter("gr8", [128, 2, 9, 2, 128], FP8E5,
                                      isOutput=False)
    q1_d = nc.declare_dram_parameter("q1s", [128, 2], F32, isOutput=False)
    ob_d = nc.declare_dram_parameter("obs", [128, 2], F32, isOutput=False)
    owt_d = nc.declare_dram_parameter("owt", [128, 2, 256], BF16,
                                      isOutput=False)
    awp_d = nc.declare_dram_parameter("awp", [128, 2, 3], BF16, isOutput=False)
    dwp_d = nc.declare_dram_parameter("dwp", [128, 2, 2, 10], F32,
                                      isOutput=False)
    dg5_d = nc.declare_dram_parameter("dg5", [128, 9, 128], BF16,
                                      isOutput=False)
    out_d = nc.declare_dram_parameter("out", [PB, C, H, W], F32, isOutput=True)

    with tile.TileContext(nc) as tc:
        _body(nc, tc, xpe_d, x8_d, xr8_d, g8_d, gr8_d, q1_d, ob_d, owt_d,
              awp_d, dwp_d, dg5_d, out_d, u_vals, reps)
    nc.finalize()
    return nc


def _body(nc, tc, xpe_d, x8_d, xr8_d, g8_d, gr8_d, q1_d, ob_d, owt_d, awp_d,
          dwp_d, dg5_d, out_d, u_vals, reps):
    ctx = ExitStack()
    with ctx:
        singles = ctx.enter_context(tc.tile_pool(name="singles", bufs=1))
        xpep = ctx.enter_context(tc.tile_pool(name="xpep", bufs=2))
        x8p = ctx.enter_context(tc.tile_pool(name="x8p", bufs=2))
        xr8p = (ctx.enter_context(tc.tile_pool(name="xr8p", bufs=1))
                if XCOMP_TAPS else None)
        a1p = ctx.enter_context(tc.tile_pool(name="a1p", bufs=1))
        accp = ctx.enter_context(tc.tile_pool(name="accp", bufs=2))
        prods = ctx.enter_context(tc.tile_pool(name="prods", bufs=2))
        smalls = ctx.enter_context(tc.tile_pool(name="smalls", bufs=1))
        gwp = ctx.enter_context(tc.tile_pool(name="gwp", bufs=15))
        ostp = ctx.enter_context(tc.tile_pool(name="ostp", bufs=3))
        dramp = ctx.enter_context(tc.tile_pool(name="dramp", bufs=2,
                                               space="DRAM"))
        ps_l = ctx.enter_context(tc.tile_pool(name="ps_l", bufs=1,
                                              space="PSUM"))
        ps_t = ctx.enter_context(tc.tile_pool(name="ps_t", bufs=1,
                                              space="PSUM"))
        psg = ctx.enter_context(tc.tile_pool(name="psg", bufs=2,
                                             space="PSUM"))
        psa = ctx.enter_context(tc.tile_pool(name="psa", bufs=2,
                                             space="PSUM"))

        # ---- resident weights ----
        g8_sb = singles.tile([128, 2, 9, 2, 128], FP8)
        nc.gpsimd.dma_start(out=g8_sb, in_=g8_d[:, :, :, :, :])
        gr8_sb = singles.tile([128, 2, 9, 2, 128], FP8E5)
        nc.gpsimd.dma_start(out=gr8_sb, in_=gr8_d[:, :, :, :, :])
        q1_sb = singles.tile([128, 2], F32)
        nc.sync.dma_start(out=q1_sb, in_=q1_d[:, :])
        ob_sb = singles.tile([128, 2], F32)
        nc.sync.dma_start(out=ob_sb, in_=ob_d[:, :])
        owt_sb = singles.tile([128, 2, 256], BF16)
        nc.gpsimd.dma_start(out=owt_sb, in_=owt_d[:, :, :])
        awp_sb = singles.tile([128, 2, 3], BF16)
        nc.sync.dma_start(out=awp_sb, in_=awp_d[:, :, :])
        dwp_sb = singles.tile([128, 2, 2, 10], F32)
        nc.gpsimd.dma_start(out=dwp_sb, in_=dwp_d[:, :, :, :])
        dg5_sb = singles.tile([128, 9, 128], BF16)
        nc.gpsimd.dma_start(out=dg5_sb, in_=dg5_d[:, :, :])
        ident = singles.tile([128, 128], F32)
        make_identity(nc, ident)

        def w_ap(ck, ki, t):
            return dwp_sb[:, ck, ki, t : t + 1]

        def b_ap(ck, ki):
            return dwp_sb[:, ck, ki, 9:10]

        st = [dict() for _ in range(PB)]

        # ---------------- emitters ----------------
        def emit_loads(img, defer_xr8=False, defer_x8=False):
            s = st[img]
            s["xpe"] = [None, None]
            s["xv"] = [None, None]
            for ck in (0, 1):
                t_ = xpep.tile([128, FWE], BF16, tag=f"xpe{ck}",
                               name=f"xpe{ck}")
                # ck1 frame loads via the idle ACT queue at startup so the
                # two xpe transfers issue in parallel
                dq = nc.sync if ck == 0 else nc.scalar
                dq.dma_start(out=t_, in_=xpe_d[img, ck, :, :])
                s["xpe"][ck] = t_
                s["xv"][ck] = t_[:, 0:FW].rearrange("p (a b) -> p a b", b=WP)
            if not defer_x8:
                s["x8"] = x8p.tile([128, 2, FW], FP8, tag="x8", name="x8t")
                nc.sync.dma_start(out=s["x8"], in_=x8_d[img, :, :, :])
            if XCOMP_TAPS and not defer_xr8:
                pass
            if XCOMP_TAPS and not defer_xr8:
                s["xr8"] = xr8p.tile([128, 2, FW], FP8, tag="xr8", name="xr8t")
                nc.gpsimd.dma_start(out=s["xr8"], in_=xr8_d[img, :, :, :])

        def emit_x8_sp(img):
            s = st[img]
            s["x8"] = x8p.tile([128, 2, FW], FP8, tag="x8", name="x8t")
            nc.sync.dma_start(out=s["x8"], in_=x8_d[img, :, :, :])

        def emit_xr8_sp(img):
            # deferred second-image residual load on the SP queue: with
            # xr8p bufs=1 it blocks until the previous image's chains
            # release the tile, so it must not sit at the head of a busy
            # engine's stream
            s = st[img]
            if XCOMP_TAPS:
                s["xr8"] = xr8p.tile([128, 2, FW], FP8, tag="xr8", name="xr8t")
                nc.sync.dma_start(out=s["xr8"], in_=xr8_d[img, :, :, :])

        def emit_logits(img):
            # one padded-frame ROW per tile so the stationary AP has a
            # single free dim (walrus requirement); logits land as
            # [64(hw-part), (j,k)]
            s = st[img]
            lps = ps_l.tile([64, 192], F32, tag="lps")
            for j in range(64):
                for ck in range(2):
                    nc.tensor.matmul(
                        lps[:, 3 * j : 3 * j + 3],
                        lhsT=s["xv"][ck][:, PAD + j, PAD : PAD + W],
                        rhs=awp_sb[:, ck, :],
                        start=(ck == 0),
                        stop=(ck == 1),
                    )
            s["lps"] = lps

        def emit_softmax(img):
            s = st[img]
            esb = smalls.tile([64, 192], F32, tag="esb")
            nc.scalar.activation(esb, s["lps"][:, :], ActFn.Exp)
            e3 = esb.rearrange("p (j k) -> p j k", k=3)
            ssum = smalls.tile([64, 64], F32, tag="ssum")
            nc.vector.tensor_scalar(
                out=ssum, in0=e3[:, :, 0], scalar1=float(u_vals[0]),
                scalar2=None, op0=AluOp.mult,
            )
            for k in (1, 2):
                nc.vector.scalar_tensor_tensor(
                    out=ssum, in0=e3[:, :, k], scalar=float(u_vals[k]),
                    in1=ssum, op0=AluOp.mult, op1=AluOp.add,
                )
            rsum = smalls.tile([64, 64], F32, tag="rsum")
            nc.vector.reciprocal_approx_fast(rsum, ssum)
            a1t = smalls.tile([64, 192], F32, tag="a1t")  # cols k*64+j
            a1t3 = a1t.rearrange("p (k j) -> p k j", j=64)
            for k in range(3):
                nc.vector.scalar_tensor_tensor(
                    out=a1t3[:, k, :], in0=e3[:, :, k],
                    scalar=float(u_vals[k]), in1=ssum,
                    op0=AluOp.mult, op1=AluOp.add,
                )
                nc.vector.tensor_tensor(
                    out=a1t3[:, k, :], in0=a1t3[:, k, :], in1=rsum,
                    op=AluOp.mult,
                )
            s["a1t"] = a1t

        def emit_transpose(img):
            # [64, 192] -> two [96, 64] transposes; rows stay (k*64 + j)
            s = st[img]
            tps = ps_t.tile([96, 2, 64], F32, tag="tps")
            for h in range(2):
                nc.tensor.transpose(tps[:, h, :],
                                    s["a1t"][:, 96 * h : 96 * h + 96],
                                    ident[0:64, 0:64])
            a1rows = smalls.tile([96, 2, 64], BF16, tag="a1rows")
            nc.vector.tensor_copy(a1rows, tps[:, :, :])
            s["a1rows"] = a1rows

        def emit_a1dma(img, ks=(1, 0, 2), write=True):
            s = st[img]
            if write:
                a1dram = dramp.tile([2, 96, 64], BF16, tag="a1dram")
                nc.sync.dma_start(
                    out=a1dram.rearrange("h r w -> r h w"), in_=s["a1rows"])
                s["a1dram"] = a1dram
                s["a1sb"] = [None, None, None]
            a1dram = s["a1dram"]
            for k in ks:
                a1k = a1p.tile([128, HW], BF16, tag=f"a1{k}")
                bsrc = bass.AP(
                    tensor=a1dram.tensor,
                    offset=a1dram.offset + k * HW,
                    ap=[[0, 128], [1, HW]],
                )
                nc.sync.dma_start(out=a1k, in_=bsrc)
                s["a1sb"][k] = a1k

        def emit_chain(img, ok, nb):
            s = st[img]
            xcomp = XCOMP_TAPS if img == 0 else ()
            if nb % 2 == 0:
                pgpair = psg.tile([128, 2, 512], F32, tag="pg", name="pgpair")
                s["pg"][(ok, nb // 2)] = pgpair
            pg = s["pg"][(ok, nb // 2)][:, nb % 2, :]
            h0 = nb * BR
            x8 = s["x8"]
            # row-granular 64-col runs skip the 10 pad cols per row that
            # the old 296-col runs computed and discarded (13.5% of PE)
            first = True
            for wt, t in [(g8_sb, t) for t in range(9)] + [
                    (gr8_sb, t) for t in range(9)]:
                dy, dx = (t // 3 - 1) * 2, (t % 3 - 1) * 2
                base = (PAD + h0 + dy) * WP + PAD + dx
                for r in range(BR):
                    nc.tensor.matmul(
                        pg[:, r * W : r * W + W], lhsT=wt[:, :, t, ok, :],
                        rhs=x8[:, :, base + r * WP : base + r * WP + W],
                        start=first,
                        stop=(wt is gr8_sb and t == 8 and r == BR - 1),
                        perf_mode=PerfMode.DoubleRow,
                    )
                    first = False

        def emit_diag_chain(img, nb):
            # branch d=5 ck=1 as bf16 diagonal matmuls: 9 taps accumulate
            # conv5(ck1) rows [nb*4, nb*4+4) in PSUM; shares the psg ring
            s = st[img]
            if nb % 2 == 0:
                pdpair = psg.tile([128, 2, 512], F32, tag="pg",
                                  name="pdpair")
                s["pd"][nb // 2] = pdpair
            pd = s["pd"][nb // 2][:, nb % 2, :]
            h0 = nb * BR
            xf = s["xpe"][1]
            for t in range(9):
                dy, dx = (t // 3 - 1) * 5, (t % 3 - 1) * 5
                base = (PAD + h0 + dy) * WP + PAD + dx
                for r in range(BR):
                    nc.tensor.matmul(
                        pd[0:128, r * W : r * W + W], lhsT=dg5_sb[:, t, :],
                        rhs=xf[:, base + r * WP : base + r * WP + W],
                        start=(t == 0 and r == 0),
                        stop=(t == 8 and r == BR - 1),
                    )

        def emit_diag_drain(img, nbp):
            # one DVE pass: m5ck1 rows = (conv5 + b5) * (1+attn_2)
            s = st[img]
            pd = s["pd"].pop(nbp)
            acc = s["accs"][(1, 1)]
            for j in (0, 1):
                r0 = nbp * 8 + j * BR
                nc.vector.scalar_tensor_tensor(
                    out=acc[:, r0 : r0 + BR, :].rearrange(
                        "p r q -> p (r q)"),
                    in0=pd[:, j, 0 : BR * W],
                    scalar=b_ap(1, 1),
                    in1=s["a1sb"][2][:, r0 * W : r0 * W + BR * W],
                    op0=AluOp.add, op1=AluOp.mult,
                )
            s["dg_done"] = s.get("dg_done", 0) + 1

        def emit_gw(img, ok, nbp, eng="act"):
            # paired PSUM drain (GPSIMD may not access PSUM on hardware):
            # gw = P_g/64 + q1 for two 4-row blocks in one op; the
            # (1+attn_1) factor is applied on gpsimd at combine time
            s = st[img]
            pg = s["pg"].pop((ok, nbp))
            gw = gwp.tile([128, 2, BR * W], BF16, tag="gw")
            pgv = pg[:, :, 0 : BR * W]
            if eng == "act":
                nc.scalar.activation(gw.rearrange("p b f -> p (b f)"),
                                     pgv, ActFn.Identity,
                                     bias=q1_sb[:, ok : ok + 1], scale=GINV)
            else:
                nc.vector.tensor_scalar(
                    out=gw.rearrange("p b f -> p (b f)"), in0=pgv,
                    scalar1=GINV, scalar2=q1_sb[:, ok : ok + 1],
                    op0=AluOp.mult, op1=AluOp.add,
                )
            s["gw"][(ok, nbp)] = gw

        def emit_pa(img, ok, nbp):
            # one PSUM bank covers TWO 4-row blocks (8 rows, 512 cols):
            # halves psa ring turns and combine op count
            s = st[img]
            off = nbp * 2 * BR * W
            pa = psa.tile([128, 2 * BR * W], F32, tag="pa")
            firstmm = True
            for ck in range(2):
                for ki in range(2):
                    nc.tensor.matmul(
                        pa[:, 0 : 2 * BR * W],
                        lhsT=owt_sb[:, ck, ok * 128 : ok * 128 + 128],
                        rhs=s["m"][(ki, ck)][:, off : off + 2 * BR * W],
                        start=firstmm,
                        stop=(ck == 1 and ki == 1),
                    )
                    firstmm = False
            s["pa"][(ok, nbp)] = pa

        def emit_osb(img, ok, nbp, eng="gp"):
            s = st[img]
            ost = ostp.tile([128, HW // 8], F32, tag="ost", name="ost")
            pa = s["pa"].pop((ok, nbp))
            gw = s["gw"].pop((ok, nbp))
            poff = nbp * 2 * BR * W
            gwf = gw.rearrange("p b f -> p (b f)")
            nc.gpsimd.tensor_tensor(
                out=gwf, in0=gwf,
                in1=s["a1sb"][1][:, poff : poff + 2 * BR * W],
                op=AluOp.mult,
            )
            if eng == "acta":
                # ACT drains (P_a + ob); Pool adds the weighted gw -- keeps
                # the combine off DVE during DVE-bound phases
                nc.scalar.activation(
                    ost, pa, ActFn.Identity,
                    bias=ob_sb[:, ok : ok + 1], scale=1.0)
                nc.gpsimd.tensor_tensor(out=ost, in0=ost, in1=gwf,
                                        op=AluOp.add)
            else:
                nc.vector.scalar_tensor_tensor(
                    out=ost, in0=pa, scalar=ob_sb[:, ok : ok + 1],
                    in1=gwf, op0=AluOp.add, op1=AluOp.add,
                )
            nc.sync.dma_start(
                out=out_d[img, ok * 128 : ok * 128 + 128,
                          nbp * 8 : nbp * 8 + 8, :],
                in_=ost.rearrange("p (a b) -> p a b", b=W),
            )

        def emit_merge(img, hf):
            # apply (1+attn_k) to one H/2 half of each acc; with subtile
            # deps this unlocks P_a for row blocks of that half only
            s = st[img]
            if "m" not in s:
                s["m"] = {}
            lo, hi = hf * (HW // 2), (hf + 1) * (HW // 2)
            for ki in range(2):
                a1k = s["a1sb"][0] if ki == 0 else s["a1sb"][2]
                e_ = nc.vector if ki == 0 else nc.gpsimd
                for ck in range(2):
                    av = s["accs"][(ki, ck)].rearrange("p h w -> p (h w)")
                    if DIAG and (ki, ck) == (1, 1):
                        s["m"][(ki, ck)] = av
                        continue
                    e_.tensor_tensor(out=av[:, lo:hi], in0=av[:, lo:hi],
                                     in1=a1k[:, lo:hi], op=AluOp.mult)
                    s["m"][(ki, ck)] = av


        # ---- elementwise tap op queues ----
        def tap_view(img, ck, ki, t):
            d = DILS[ki]
            dy, dx = (t // 3 - 1) * d, (t % 3 - 1) * d
            return st[img]["xv"][ck][:, PAD + dy : PAD + dy + H,
                                     PAD + dx : PAD + dx + W]

        def make_tap_queues(img):
            s = st[img]
            s["accs"] = {}
            s["pg"] = {}
            s["gw"] = {}
            s["pa"] = {}
            s["ost"] = {}
            for ki in range(2):
                for ck in range(2):
                    s["accs"][(ki, ck)] = accp.tile(
                        [128, H, W], BF16, tag=f"acc{ki}{ck}", name=f"acc{ki}{ck}")

            s["pend"] = {}

            def dve_first(ki, ck):
                def f():
                    nc.vector.tensor_scalar(
                        out=s["accs"][(ki, ck)], in0=tap_view(img, ck, ki, 0),
                        scalar1=w_ap(ck, ki, 0), scalar2=b_ap(ck, ki),
                        op0=AluOp.mult, op1=AluOp.add,
                    )
                return f

            def act_first(ki, ck):
                # ACT Identity with AP bias+scale writes w*x+b into the acc
                def f():
                    nc.scalar.activation(
                        s["accs"][(ki, ck)], tap_view(img, ck, ki, 0),
                        ActFn.Identity, bias=b_ap(ck, ki),
                        scale=w_ap(ck, ki, 0))
                return f

            HH = H // 2

            def tap_hview(ck, ki, t, hf):
                d = DILS[ki]
                dy, dx = (t // 3 - 1) * d, (t % 3 - 1) * d
                r0 = PAD + dy + hf * HH
                return s["xv"][ck][:, r0 : r0 + HH, PAD + dx : PAD + dx + W]

            def dve_prod(ki, ck, t, hf):
                # half-tile product at DVE 4x rate (tensor_scalar)
                def f():
                    p = prods.tile([128, HH, W], BF16, tag="prodd", bufs=2)
                    nc.vector.tensor_scalar(
                        out=p, in0=tap_hview(ck, ki, t, hf),
                        scalar1=w_ap(ck, ki, t), scalar2=None, op0=AluOp.mult)
                    s["pend"][(ki, ck, t, hf)] = p
                return f

            def act_prod(ki, ck, t, hf):
                def f():
                    p = prods.tile([128, HH, W], BF16, tag="proda", bufs=2)
                    nc.scalar.activation(p, tap_hview(ck, ki, t, hf),
                                         ActFn.Copy, bias=0.0,
                                         scale=w_ap(ck, ki, t))
                    s["pend"][(ki, ck, t, hf)] = p
                return f

            def gp_prod(ki, ck, t, hf):
                def f():
                    p = prods.tile([128, HH, W], BF16, tag="prodg", bufs=2)
                    nc.gpsimd.tensor_scalar(
                        out=p, in0=tap_hview(ck, ki, t, hf),
                        scalar1=w_ap(ck, ki, t), scalar2=None, op0=AluOp.mult)
                    s["pend"][(ki, ck, t, hf)] = p
                return f

            def dve_add(ki, ck, t, hf):
                def f():
                    a = s["accs"][(ki, ck)]
                    av = a[:, hf * HH : (hf + 1) * HH, :]
                    p = s["pend"].pop((ki, ck, t, hf))
                    nc.vector.tensor_tensor(out=av, in0=av, in1=p,
                                            op=AluOp.add)
                return f

            def gp_add(ki, ck, t, hf):
                def f():
                    a = s["accs"][(ki, ck)]
                    av = a[:, hf * HH : (hf + 1) * HH, :]
                    p = s["pend"].pop((ki, ck, t, hf))
                    nc.gpsimd.tensor_tensor(out=av, in0=av, in1=p,
                                            op=AluOp.add)
                return f

            PRODF = {"d": dve_prod, "a": act_prod, "g": gp_prod}
            ADDF = {"d": dve_add, "g": gp_add}
            QPICK = {"d": "dve_q", "a": "act_q", "g": "gp_q"}

            dve_q, act_q, gp_q = [], [], []
            qs = {"dve_q": dve_q, "act_q": act_q, "gp_q": gp_q}
            # entries are (need_key, fn): an add may only be emitted once its
            # product's closure has run (registered need_key in s["pend"])
            s["pd"] = {}
            for (ki, ck), eng in FIRST_ROUTE.items():
                if DIAG and (ki, ck) == (1, 1):
                    continue  # produced by the PE diag chains
                qs[QPICK[eng]].append(
                    (None, (dve_first if eng == "d" else act_first)(ki, ck)))
            # HALF-MAJOR order: all half-0 products+adds first, so the lower
            # 32 rows of every acc finish early and merge/P_a/osb for row
            # blocks nb<8 can overlap the rest of the tap phase
            for hf in (0, 1):
                for t in range(1, 9):
                    for ki in range(2):
                        for ck in range(2):
                            if DIAG and (ki, ck) == (1, 1):
                                continue
                            pe_, ae_ = TAP_ROUTE[(ki, ck, t)]
                            qs[QPICK[pe_]].append(
                                (None, PRODF[pe_](ki, ck, t, hf)))
                            qs[QPICK[ae_]].append(
                                ((ki, ck, t, hf), ADDF[ae_](ki, ck, t, hf)))
                if hf == 0:
                    s["h0len"] = {qn: len(qs[qn]) for qn in qs}
            s["gp_q"] = gp_q
            s["act_q"] = act_q
            s["dve_q"] = dve_q
            s["pops"] = {"dve_q": 0, "act_q": 0, "gp_q": 0}
            s["q0len"] = {qn: len(qs[qn]) for qn in qs}

        def h0_done(s):
            return all(
                s["q0len"][qn] - len(s[qn]) >= s["h0len"][qn]
                for qn in ("dve_q", "act_q", "gp_q"))

        def pump(s, qn, n):
            q = s[qn]
            c = 0
            while q and c < n:
                need, fn = q[0]
                if need is not None and need not in s["pend"]:
                    break  # product not yet emitted; retry next step
                q.pop(0)
                fn()
                c += 1

        def pump_flush(s):
            # round-robin flush so cross-queue product->add deps resolve
            stuck = 0
            while (s["act_q"] or s["dve_q"] or s["gp_q"]) and stuck < 3:
                before = len(s["act_q"]) + len(s["dve_q"]) + len(s["gp_q"])
                pump(s, "act_q", 99)
                pump(s, "dve_q", 99)
                pump(s, "gp_q", 99)
                after = len(s["act_q"]) + len(s["dve_q"]) + len(s["gp_q"])
                stuck = stuck + 1 if after == before else 0
            assert not (s["act_q"] or s["dve_q"] or s["gp_q"]), "queue wedged"

        # ---------------- master schedule ----------------
        def two_images(i0, i1):
            g_order = [(ok, nb) for ok in range(2) for nb in range(NBLK)]
            emit_loads(i0)
            emit_logits(i0)
            make_tap_queues(i0)
            emit_softmax(i0)
            for ok, nb in g_order[0:3]:
                emit_chain(i0, ok, nb)
            emit_transpose(i0)
            emit_loads(i1, defer_xr8=True, defer_x8=True)
            emit_a1dma(i0, ks=(1,))
            emit_x8_sp(i1)
            pump(st[i0], "act_q", 2)
            pump(st[i0], "dve_q", 3)
            pump(st[i0], "gp_q", 2)
            emit_a1dma(i0, ks=(0, 2), write=False)
            emit_logits(i1)
            emit_softmax(i1)
            for ok, nb in g_order[3:6]:
                emit_chain(i0, ok, nb)
            pump(st[i0], "act_q", 2)
            pump(st[i0], "dve_q", 3)
            pump(st[i0], "gp_q", 2)
            emit_transpose(i1)

            # Combined chain window: both images' fp8 chains interleave
            # through the taps of BOTH images, so the TensorEngine never
            # drains while ACT/DVE are saturated.  Drains (which need no
            # attention data) are emitted ahead of each step's tap work.
            make_tap_queues(i1)
            chain_q = []
            q0 = [(i0, "f") + g_order[k] for k in range(6, 32)]
            dq0 = [(i0, "g", 0, nb) for nb in range(16)] if DIAG else []
            # i0 diag blocks interleave 2:1 with i0 fp8; i1 diag goes last
            # (its drains need a1sb[2](i1), loaded mid-schedule)
            q0m = []
            while q0 or dq0:
                for _ in range(2):
                    if q0:
                        q0m.append(q0.pop(0))
                # both blocks of a diag pair stay ADJACENT so the PE stream
                # never waits on its own later instruction via the psg ring
                for _ in range(2):
                    if dq0:
                        q0m.append(dq0.pop(0))
            q1 = ([(i1, "f") + g_order[k] for k in range(32)]
                  + ([(i1, "g", 0, nb) for nb in range(16)] if DIAG else []))
            while q0m or q1:
                if q0m:
                    chain_q.append(q0m.pop(0))
                if q1:
                    chain_q.append(q1.pop(0))
            gwi = 0
            ready = [(i0, 0, 0), (i0, 0, 1), (i0, 0, 2)]
            dready = []
            for _ in range(2):
                emit_gw(*ready.pop(0), eng="dve")

            def chain_step(n):
                for _ in range(n):
                    if chain_q:
                        img, kind, ok, nb = chain_q.pop(0)
                        if kind == "f":
                            emit_chain(img, ok, nb)
                            if nb % 2 == 1:
                                ready.append((img, ok, nb // 2))
                        else:
                            emit_diag_chain(img, nb)
                            if nb % 2 == 1:
                                dready.append((img, nb // 2))

            def drain_ready(keep=0):
                nonlocal gwi
                while len(ready) > keep:
                    emit_gw(*ready.pop(0), eng=("act", "act", "dve")[gwi % 3])
                    gwi += 1
                keep_d = []
                while dready:
                    img, nbp = dready.pop(0)
                    if st[img].get("a1sb") and st[img]["a1sb"][2] is not None:
                        emit_diag_drain(img, nbp)
                    else:
                        keep_d.append((img, nbp))
                dready.extend(keep_d)

            # nb-block order for P_a/osb: half-0 blocks (nb<8) first so they
            # can start right after the half-0 merge
            h_order = ([(ok, nbp) for ok in range(2) for nbp in range(4)]
                       + [(ok, nbp) for ok in range(2) for nbp in range(4, 8)])
            paq = list(h_order)
            osq = list(h_order)
            merged = {i0: 0, i1: 0}

            def try_merge(img):
                s = st[img]
                if merged[img] == 0 and h0_done(s):
                    emit_merge(img, 0)
                    merged[img] = 1
                if (merged[img] == 1 and not s["dve_q"] and not s["act_q"]
                        and not s["gp_q"]):
                    emit_merge(img, 1)
                    merged[img] = 2

            def pump_pa_osb(img, pq, oq, npa, nos, eng=None):
                # the ACT-drain combine may only run once no chain matmuls
                # remain ahead of it on PE (else ACT stream order can cycle
                # through psg -> gw-drain -> acta)
                if eng is None:
                    eng = "gp" if chain_q else "acta"
                lim = {0: 0, 1: 8, 2: 16}[merged[img]]
                for _ in range(nos):
                    if (oq and len(oq) > len(pq)
                            and (oq[0][0], oq[0][1]) in st[img]["gw"]):
                        emit_osb(img, *oq.pop(0), eng=eng)
                for _ in range(npa):
                    done = 16 - len(pq)
                    # psa ring has 2 slots: P_a may lead its combine by <=2
                    if pq and done < lim and len(oq) - len(pq) < 2:
                        if DIAG and st[img].get("dg_done", 0) < pq[0][1] + 1:
                            break  # m(1,1) rows not yet drained from PSUM
                        emit_pa(img, *pq.pop(0))

            for step in range(17):
                chain_step(2)
                drain_ready()
                pump(st[i0], "act_q", 2)
                pump(st[i0], "dve_q", 3)
                pump(st[i0], "gp_q", 3)
                try_merge(i0)
                pump_pa_osb(i0, paq, osq, 2, 2)
            pump_flush(st[i0])
            try_merge(i0)
            # complete ALL i0 chains (incl. diag) and their drains before
            # a1dma(i1): the i1 a1k loads wait on i0's a1 releases (diag
            # drains consume a1sb[2](i0)), and i0's osb stores queue behind
            # the a1k DMAs on SP -- emitting them later would cycle
            rest0 = [e for e in chain_q if e[0] == i0]
            chain_q[:] = [e for e in chain_q if e[0] != i0]
            for idx, (img_, kind_, ok_, nb_) in enumerate(rest0):
                if kind_ == "f":
                    emit_chain(img_, ok_, nb_)
                    if nb_ % 2 == 1:
                        ready.append((img_, ok_, nb_ // 2))
                else:
                    emit_diag_chain(img_, nb_)
                    if nb_ % 2 == 1:
                        dready.append((img_, nb_ // 2))
                if idx % 2 == 1:
                    drain_ready()
            drain_ready()
            emit_a1dma(i1)
            # a1sb[2](i1) is now loading: spread i1's diag pairs among its
            # remaining fp8 pairs instead of bunching them in the PE tail
            rem_f = [e for e in chain_q if e[1] == "f"]
            rem_d = [e for e in chain_q if e[1] == "g"]
            chain_q.clear()
            while rem_f or rem_d:
                for _ in range(4):
                    if rem_f:
                        chain_q.append(rem_f.pop(0))
                for _ in range(2):
                    if rem_d:
                        chain_q.append(rem_d.pop(0))

            paq1 = list(h_order)
            osq1 = list(h_order)
            for step in range(14):
                chain_step(3)
                drain_ready()
                try_merge(i0)
                pump_pa_osb(i0, paq, osq, 2, 2)
                pump(st[i1], "act_q", 3)
                pump(st[i1], "dve_q", 5)
                pump(st[i1], "gp_q", 4)
                try_merge(i1)
                if not paq:
                    pump_pa_osb(i1, paq1, osq1, 2, 2)
            chain_step(99)
            drain_ready()
            pump_flush(st[i1])
            try_merge(i0)
            for ok, nb in paq:
                emit_pa(i0, ok, nb)
            for ok, nb in osq:
                emit_osb(i0, ok, nb)
            try_merge(i1)

            # P_a1 + osb1 drain
            di = 0
            while paq1 or osq1:
                for _ in range(2):
                    if paq1:
                        emit_pa(i1, *paq1.pop(0))
                while len(osq1) > len(paq1) and osq1:
                    emit_osb(i1, *osq1.pop(0), eng="acta")
                    di += 1

        for _ in range(reps):
            two_images(0, 1)


def make_in_maps(x, dw_w, dw_b, attn_w, attn_b, out_w, out_b):
    """Host-side packing. Returns (in_maps list for 8 cores, u_vals)."""
    import ml_dtypes

    bf16 = ml_dtypes.bfloat16
    fp8 = ml_dtypes.float8_e4m3
    fp8e5 = ml_dtypes.float8_e5m2

    x = np.asarray(x, np.float32)
    xpad = np.zeros((B, C, WP, WP), np.float32)
    xpad[:, :, PAD : PAD + H, PAD : PAD + W] = x
    xpe = np.zeros((B, 2, 128, FWE), np.float32)
    xpe[:, :, :, :FW] = xpad.reshape(B, 2, 128, FW)
    xpe = np.ascontiguousarray(xpe.astype(bf16))  # [B, ck, c, FWE]
    x8 = xpad.astype(fp8)
    xr = xpad - x8.astype(np.float32)
    xr8 = xr.astype(fp8)
    # [B, c(128), ck, FW] so per-image one DMA fills [128, 2, FW]
    x8_l = np.ascontiguousarray(
        x8.reshape(B, 2, 128, FW).transpose(0, 2, 1, 3)
    )
    xr8_l = np.ascontiguousarray(
        xr8.reshape(B, 2, 128, FW).transpose(0, 2, 1, 3)
    )

    w1 = dw_w[1].reshape(C, 9)  # [c, t]
    g = (out_w.T[:, None, :] * w1[:, :, None]) * GSCALE  # [c, t, o]
    g8 = g.astype(fp8)
    gr8 = (g - g8.astype(np.float32)).astype(fp8e5)
    # layout [c_lo(128), ck, t, ok, o_lo(128)]
    g8_l = np.ascontiguousarray(
        g8.reshape(2, 128, 9, 2, 128).transpose(1, 0, 2, 3, 4)
    )
    gr8_l = np.ascontiguousarray(
        gr8.reshape(2, 128, 9, 2, 128).transpose(1, 0, 2, 3, 4)
    )
    q1 = (out_w.astype(np.float64) @ dw_b[1].astype(np.float64)).astype(
        np.float32
    )
    q1s = np.ascontiguousarray(q1.reshape(2, 128).T)
    obs = np.ascontiguousarray(out_b.reshape(2, 128).T.astype(np.float32))
    owt = np.ascontiguousarray(
        np.stack([out_w.T[:128], out_w.T[128:]], axis=1)
    ).astype(bf16)  # [c_lo, ck, o]
    awp = np.ascontiguousarray(
        np.stack([attn_w.T[:128], attn_w.T[128:]], axis=1)
    ).astype(bf16)  # [c_lo, ck, k]
    # dwp: [c_lo, ck, ki(0:d=1,1:d=5), 10]
    dwp = np.zeros((128, 2, 2, 10), np.float32)
    for ki, kk in ((0, 0), (1, 2)):
        wk = dw_w[kk].reshape(C, 9)
        for ck in range(2):
            cs = slice(ck * 128, ck * 128 + 128)
            dwp[:, ck, ki, :9] = wk[cs]
            dwp[:, ck, ki, 9] = dw_b[kk][cs]
    w5t = dw_w[2].reshape(C, 9)[128:]  # [c_lo of ck1, t]
    dg5 = np.zeros((128, 9, 128), np.float32)
    dg5[np.arange(128), :, np.arange(128)] = w5t
    dg5 = dg5.astype(bf16)
    u_vals = np.exp(attn_b.astype(np.float64)).astype(np.float32)

    in_maps = []
    for i in range(N_CORES):
        sl = slice(i * PB, (i + 1) * PB)
        in_maps.append({
            "xpe": np.ascontiguousarray(xpe[sl]),
            "x8": np.ascontiguousarray(x8_l[sl]),
            "xr8": np.ascontiguousarray(xr8_l[sl]),
            "g8": g8_l,
            "gr8": gr8_l,
            "q1s": q1s,
            "obs": obs,
            "owt": owt,
            "awp": awp,
            "dwp": dwp,
            "dg5": dg5,
        })
    return in_maps, u_vals


def kernel(**inputs) -> np.ndarray:
    in_maps, u_vals = make_in_maps(
        np.asarray(inputs["x"], np.float32),
        np.asarray(inputs["dw_w"], np.float32),
        np.asarray(inputs["dw_b"], np.float32),
        np.asarray(inputs["attn_w"], np.float32),
        np.asarray(inputs["attn_b"], np.float32),
        np.asarray(inputs["out_w"], np.float32),
        np.asarray(inputs["out_b"], np.float32),
    )
    nc = build_bass(u_vals)

    from concourse.bass_utils import run_bass_kernel_spmd

    res = run_bass_kernel_spmd(nc, in_maps, core_ids=list(range(N_CORES)))
    outs = [res.results[i]["out"] for i in range(N_CORES)]
    return np.concatenate(outs, axis=0).astype(np.float32)


if __name__ == "__main__":
    nc = build_bass([1.0, 1.0, 1.0])
    print("built ok")

